# revision 3
# baseline (speedup 1.0000x reference)
"""Trainium2 Bass kernel for nn_CTGCalibratedBinary.

Computes y = x @ (sign * expand64(relu(block_scales) + 1e-6)) for
x:(8192,4096), sign:(4096,4096), block_scales:(64,64), all fp32.

Sharding (8 cores): 2 token-groups x 4 out-col-groups.
  core c: r = c // 4 (token half), q = c % 4 (col quarter)
  per-core problem: y_c[4096, 1024] = x_r[4096, 4096] @ w[:, q*1024:(q+1)*1024]

Per-core kernel strategy:
  - x is passed pre-transposed (xt = x_r.T, [K=4096, M=4096]) and declared
    float32r: the PE rounds fp32 -> 11-bit mantissa on ingest (measured), so
    matmuls run at full (bf16) speed with ~1.5e-4 L2 error, no cast passes.
  - sign shard is streamed once, dequantized on DVE into a fully SBUF-resident
    fp32r weight [128, 32, 1024] (128 KiB/partition).
  - block scales: mag = relu(bs)+1e-6 on DVE, expanded to a [128, 32*16] tile
    via a broadcast DMA through a DRAM scratch.
  - main loop over 32 m-tiles: one 2 MiB DMA brings xt[:, mt*128:+128] as
    [128, 32, 128] (k-major), then 64 matmuls accumulate into 2 PSUM banks.
"""
import os
import sys
import time

for _p in ("/opt/trn_rl_repo",):
    if _p not in sys.path and os.path.isdir(_p):
        sys.path.insert(0, _p)

import numpy as np

TOKENS = 8192
N_IN = 4096
N_OUT = 4096
BLOCK = 64

N_CORES = 8
R_GROUPS = 2          # token groups
Q_GROUPS = 4          # out-col groups
M_SHARD = TOKENS // R_GROUPS      # 4096
N_SHARD = N_OUT // Q_GROUPS       # 1024
NB_SHARD = N_SHARD // BLOCK       # 16 col-blocks per core
K_TILES = N_IN // 128             # 32
M_TILES = M_SHARD // 128          # 32

_RUNNER = None


def _build_module():
    import concourse.mybir as mybir
    import concourse.tile as tile
    from concourse import bacc

    dt = mybir.dt
    nc = bacc.Bacc("TRN2", target_bir_lowering=False, debug=False,
                   num_devices=N_CORES)

    xt = nc.dram_tensor("xt", [N_IN, M_SHARD], dt.float32r, kind="ExternalInput")
    sgn = nc.dram_tensor("sgn", [N_IN, N_SHARD], dt.float32, kind="ExternalInput")
    bs = nc.dram_tensor("bs", [BLOCK, NB_SHARD], dt.float32, kind="ExternalInput")
    y = nc.dram_tensor("y", [M_SHARD, N_SHARD], dt.float32, kind="ExternalOutput")

    with tile.TileContext(nc) as tc:
        with tc.tile_pool(name="const", bufs=1) as const_pool, \
             tc.tile_pool(name="dram", bufs=1, space="DRAM") as dram_pool, \
             tc.tile_pool(name="w", bufs=1) as w_pool, \
             tc.tile_pool(name="sgn", bufs=3) as sgn_pool, \
             tc.tile_pool(name="x", bufs=2) as x_pool, \
             tc.tile_pool(name="o", bufs=3) as o_pool, \
             tc.tile_pool(name="ps", bufs=2, space="PSUM") as ps_pool:

            # --- scales: mag = relu(bs) + 1e-6, expanded to s_full[p, kt*16+b]
            bs_t = const_pool.tile([BLOCK, NB_SHARD], dt.float32)
            nc.sync.dma_start(bs_t[:], bs.ap())
            mag_t = const_pool.tile([BLOCK, NB_SHARD], dt.float32)
            nc.vector.tensor_scalar(
                out=mag_t[:], in0=bs_t[:],
                scalar1=0.0, scalar2=1e-6,
                op0=mybir.AluOpType.max, op1=mybir.AluOpType.add,
            )
            mag_d = dram_pool.tile([BLOCK, NB_SHARD], dt.float32)
            nc.sync.dma_start(mag_d[:], mag_t[:])
            # s_full[r2*64+p, kt*16+b] = mag[2*kt+r2, b]
            s_full = const_pool.tile([128, K_TILES * NB_SHARD], dt.float32)
            mag_3d = mag_d[:].rearrange("(kt r2) b -> kt r2 b", r2=2)
            for r2 in range(2):
                nc.sync.dma_start(
                    s_full[r2 * 64:(r2 + 1) * 64, :].rearrange(
                        "p (kt b) -> p kt b", b=NB_SHARD),
                    mag_3d[:, r2, :].unsqueeze(0).broadcast_to(
                        [64, K_TILES, NB_SHARD]),
                )

            # --- weights: stream sign, dequantize into resident fp32r tile
            w_all = w_pool.tile([128, K_TILES, N_SHARD], dt.float32r)
            for kt in range(K_TILES):
                st = sgn_pool.tile([128, N_SHARD], dt.float32)
                nc.sync.dma_start(st[:], sgn.ap()[kt * 128:(kt + 1) * 128, :])
                nc.vector.tensor_tensor(
                    out=w_all[:, kt, :].rearrange("p (b c) -> p b c", c=BLOCK),
                    in0=st[:].rearrange("p (b c) -> p b c", c=BLOCK),
                    in1=s_full[:, kt * NB_SHARD:(kt + 1) * NB_SHARD]
                        .unsqueeze(2).broadcast_to([128, NB_SHARD, BLOCK]),
                    op=mybir.AluOpType.mult,
                )

            # --- main loop
            xt_view = xt.ap().rearrange("(kt p) m -> p kt m", p=128)
            for mt in range(M_TILES):
                xs = x_pool.tile([128, K_TILES, 128], dt.float32r)
                nc.sync.dma_start(
                    xs[:], xt_view[:, :, mt * 128:(mt + 1) * 128])
                ps = ps_pool.tile([128, 2, 512], dt.float32)
                for kt in range(K_TILES):
                    for j in range(2):
                        nc.tensor.matmul(
                            ps[:, j, :],
                            xs[:, kt, :],
                            w_all[:, kt, j * 512:(j + 1) * 512],
                            start=(kt == 0), stop=(kt == K_TILES - 1),
                        )
                ot = o_pool.tile([128, N_SHARD], dt.float32)
                nc.scalar.copy(
                    out=ot[:].rearrange("p (j n) -> p j n", j=2), in_=ps[:])
                nc.sync.dma_start(y.ap()[mt * 128:(mt + 1) * 128, :], ot[:])

    nc.compile()
    return nc


class _Runner:
    """Persistent compiled SPMD executable over the 8 axon cores."""

    def __init__(self):
        import jax
        import jax.numpy as jnp
        from jax.sharding import Mesh, PartitionSpec
        from jax.experimental.shard_map import shard_map
        import concourse.mybir as mybir
        from concourse import bass2jax

        self.jax = jax
        nc = _build_module()
        self.nc = nc
        bass2jax.install_neuronx_cc_hook()

        partition_name = (nc.partition_id_tensor.name
                          if nc.partition_id_tensor else None)
        in_names = []
        out_names = []
        out_avals = []
        zero_outs = []
        for alloc in nc.m.functions[0].allocations:
            if not isinstance(alloc, mybir.MemoryLocationSet):
                continue
            name = alloc.memorylocations[0].name
            if alloc.kind == "ExternalInput":
                if name == partition_name:
                    continue
                in_names.append(name)
            elif alloc.kind == "ExternalOutput":
                out_names.append(name)
                shape = tuple(alloc.tensor_shape)
                dtype = mybir.dt.np(alloc.dtype)
                out_avals.append(jax.core.ShapedArray(shape, dtype))
                zero_outs.append(np.zeros(shape, dtype))
        self.in_names = list(in_names)
        self.out_names = out_names
        self.out_avals = out_avals
        n_params = len(in_names)
        all_names = in_names + out_names
        if partition_name is not None:
            all_names = all_names + [partition_name]

        def _body(*args):
            operands = list(args)
            if partition_name is not None:
                operands.append(bass2jax.partition_id_tensor())
            outs = bass2jax._bass_exec_p.bind(
                *operands,
                out_avals=tuple(out_avals),
                in_names=tuple(all_names),
                out_names=tuple(out_names),
                lowering_input_output_aliases=(),
                sim_require_finite=True,
                sim_require_nnan=True,
                nc=nc,
            )
            return tuple(outs)

        devices = jax.devices()[:N_CORES]
        self.mesh = Mesh(np.asarray(devices), ("core",))
        n_outs = len(out_names)
        in_specs = (PartitionSpec("core"),) * (n_params + n_outs)
        out_specs = (PartitionSpec("core"),) * n_outs
        self._fn = jax.jit(
            shard_map(_body, mesh=self.mesh, in_specs=in_specs,
                      out_specs=out_specs, check_rep=False),
            keep_unused=True,
        )
        self.zero_outs = zero_outs
        self._zero_dev = None

    def put_inputs(self, in_maps):
        """Device-put concatenated per-core inputs; returns list of jax arrays."""
        from jax.sharding import NamedSharding, PartitionSpec
        sh = NamedSharding(self.mesh, PartitionSpec("core"))
        args = []
        for name in self.in_names:
            cat = np.concatenate([m[name] for m in in_maps], axis=0)
            args.append(self.jax.device_put(cat, sh))
        if self._zero_dev is None:
            self._zero_dev = [
                self.jax.device_put(
                    np.zeros((N_CORES * z.shape[0], *z.shape[1:]), z.dtype), sh)
                for z in self.zero_outs
            ]
        return args + self._zero_dev

    def run(self, args):
        outs = self._fn(*args)
        self.jax.block_until_ready(outs)
        return outs

    def split_outputs(self, outs):
        res = []
        for c in range(N_CORES):
            m = {}
            for i, name in enumerate(self.out_names):
                shape = self.out_avals[i].shape
                m[name] = np.asarray(outs[i]).reshape(
                    N_CORES, *shape)[c]
            res.append(m)
        return res


def get_runner():
    global _RUNNER
    if _RUNNER is None:
        _RUNNER = _Runner()
    return _RUNNER


def make_in_maps(x, sign, block_scales):
    x = np.ascontiguousarray(x, dtype=np.float32)
    sign = np.ascontiguousarray(sign, dtype=np.float32)
    block_scales = np.ascontiguousarray(block_scales, dtype=np.float32)
    xt_halves = [
        np.ascontiguousarray(x[r * M_SHARD:(r + 1) * M_SHARD, :].T)
        for r in range(R_GROUPS)
    ]
    sgn_q = [
        np.ascontiguousarray(sign[:, q * N_SHARD:(q + 1) * N_SHARD])
        for q in range(Q_GROUPS)
    ]
    bs_q = [
        np.ascontiguousarray(block_scales[:, q * NB_SHARD:(q + 1) * NB_SHARD])
        for q in range(Q_GROUPS)
    ]
    in_maps = []
    for c in range(N_CORES):
        r, q = c // Q_GROUPS, c % Q_GROUPS
        in_maps.append({"xt": xt_halves[r], "sgn": sgn_q[q], "bs": bs_q[q]})
    return in_maps


def assemble(per_core_y):
    y = np.empty((TOKENS, N_OUT), dtype=np.float32)
    for c in range(N_CORES):
        r, q = c // Q_GROUPS, c % Q_GROUPS
        y[r * M_SHARD:(r + 1) * M_SHARD,
          q * N_SHARD:(q + 1) * N_SHARD] = per_core_y[c]
    return y


def kernel(x, sign, block_scales):
    runner = get_runner()
    in_maps = make_in_maps(x, sign, block_scales)
    args = runner.put_inputs(in_maps)
    outs = runner.run(args)
    per_core = runner.split_outputs(outs)
    return assemble([m["y"] for m in per_core])


if __name__ == "__main__":
    rng = np.random.default_rng(0)
    x = rng.standard_normal((TOKENS, N_IN), dtype=np.float32)
    sign = np.where(rng.standard_normal((N_IN, N_OUT)) >= 0, 1.0, -1.0).astype(np.float32)
    bs = rng.uniform(0.1, 1.0, (BLOCK, BLOCK)).astype(np.float32)
    t0 = time.perf_counter()
    out = kernel(x=x, sign=sign, block_scales=bs)
    print(f"kernel() wall: {time.perf_counter() - t0:.1f}s, out shape {out.shape}")
    mag = np.maximum(bs, 0) + 1e-6
    w = sign * np.repeat(np.repeat(mag, BLOCK, 0), BLOCK, 1)
    ref = x @ w
    l2 = np.linalg.norm(out - ref) / np.linalg.norm(ref)
    print(f"l2_rel vs fp32 numpy: {l2:.3e}")


# revision 46
# speedup vs baseline: 1.9291x; 1.9291x over previous
"""Trainium2 Bass kernel for nn_CTGCalibratedBinary.

Computes y = x @ (sign * expand64(relu(block_scales) + 1e-6)) for
x:(8192,4096), sign:(4096,4096), block_scales:(64,64), all fp32.

Sharding (8 cores): 2 token-groups x 4 out-col-groups.
  core c: r = c // 4 (token half), q = c % 4 (col quarter)
  per-core problem: y_c[4096, 1024] = x_r[4096, 4096] @ w[:, q*1024:(q+1)*1024]

Per-core kernel strategy (measured ~613 us/core steady-state on HW):
  - x is passed pre-transposed (xt = x_r.T, [K=4096, M=4096]) and declared
    float32r: the PE rounds fp32 -> 11-bit mantissa (RNE) on ingest
    (measured via identity-matmul probes), so matmuls run at full rate with
    ~1.4e-4 L2 error and no cast/rounding passes are needed anywhere.
  - sign shard is streamed once and dequantized on the vector engine into a
    fully SBUF-resident fp32r weight w_all[128, 32, 1024] (128 KiB/partition);
    the 64x64-block scales enter via a broadcast access pattern (stride-0
    inner dim), so dequant is a single tensor_tensor multiply per k-tile.
  - block scales: mag = relu(bs)+1e-6 fused in one tensor_scalar (max, add),
    then expanded to s_full[p, kt*16+b] = mag[2kt + p//64, b] with two
    broadcast DMAs through a DRAM scratch.
  - main loop over 32 m-tiles: x arrives as four [128, 8, 128] k-group tiles
    (finer DMA/dependency granularity), 64 matmuls accumulate K=4096 into
    2 PSUM banks (N=512 each, the fp32 moving-operand cap), ACT engine
    copies PSUM->SBUF, DMA out.

Perf notes from the measurement campaign (loop-in-NEFF differencing, since
axon wall-clock has ~30 ms transport noise): PE stream floor for the 2048
matmuls is ~530 us (259 ns/MM vs 213 ideal; the overhead is per-MM issue/
self-load cost, identical for bf16), x-streaming adds ~80 us (about half
pure HBM/port interference, half scheduling). Probed and rejected: psum-bank
alternation removal, stationary reuse ordering, --enable-ldw-opt=true,
transposed-output orientation (psum 8-bank cycling is worse), 1KB-line
slabs, DMA queue splitting, deeper prefetch. bf16 inputs are no faster on
this pattern and 16x less accurate; plain fp32 is 4x slower.
"""
import os
import sys
import time

for _p in ("/opt/trn_rl_repo",):
    if _p not in sys.path and os.path.isdir(_p):
        sys.path.insert(0, _p)

import numpy as np

TOKENS = 8192
N_IN = 4096
N_OUT = 4096
BLOCK = 64

N_CORES = 8
R_GROUPS = 2          # token groups
Q_GROUPS = 4          # out-col groups
M_SHARD = TOKENS // R_GROUPS      # 4096
N_SHARD = N_OUT // Q_GROUPS       # 1024
NB_SHARD = N_SHARD // BLOCK       # 16 col-blocks per core
K_TILES = N_IN // 128             # 32
M_TILES = M_SHARD // 128          # 32
XG = 8                            # k-tiles per x dependency-group tile
PS_BUFS = 4                       # psum tiles in flight (2 banks each)

_RUNNER = None


def _build_module(reps: int = 1):
    """Build the per-core Bass module. reps>1 wraps the body in a hardware
    For_i loop (identical iterations) -- used only for timing measurements."""
    import contextlib

    import concourse.mybir as mybir
    import concourse.tile as tile
    from concourse import bacc

    dt = mybir.dt
    nc = bacc.Bacc("TRN2", target_bir_lowering=False, debug=False,
                   num_devices=N_CORES)

    xt = nc.dram_tensor("xt", [N_IN, M_SHARD], dt.float32r, kind="ExternalInput")
    sgn = nc.dram_tensor("sgn", [N_IN, N_SHARD], dt.float32, kind="ExternalInput")
    bs = nc.dram_tensor("bs", [BLOCK, NB_SHARD], dt.float32, kind="ExternalInput")
    y = nc.dram_tensor("y", [M_SHARD, N_SHARD], dt.float32, kind="ExternalOutput")

    with tile.TileContext(nc) as tc:
        loop_ctx = (tc.For_i(0, reps, 1, hint_engines=(mybir.EngineType.PE,))
                    if reps > 1 else contextlib.nullcontext())
        with loop_ctx, \
             tc.tile_pool(name="const", bufs=1) as const_pool, \
             tc.tile_pool(name="dram", bufs=1, space="DRAM") as dram_pool, \
             tc.tile_pool(name="w", bufs=1) as w_pool, \
             tc.tile_pool(name="sgn", bufs=3) as sgn_pool, \
             tc.tile_pool(name="x", bufs=2 * (K_TILES // XG)) as x_pool, \
             tc.tile_pool(name="o", bufs=3) as o_pool, \
             tc.tile_pool(name="ps", bufs=PS_BUFS, space="PSUM") as ps_pool:

            # --- scales: mag = relu(bs) + 1e-6, expanded so that
            #     s_full[r2*64+p, kt*16+b] = mag[2*kt+r2, b]
            bs_t = const_pool.tile([BLOCK, NB_SHARD], dt.float32)
            nc.sync.dma_start(bs_t[:], bs.ap())
            mag_t = const_pool.tile([BLOCK, NB_SHARD], dt.float32)
            nc.vector.tensor_scalar(
                out=mag_t[:], in0=bs_t[:],
                scalar1=0.0, scalar2=1e-6,
                op0=mybir.AluOpType.max, op1=mybir.AluOpType.add,
            )
            mag_d = dram_pool.tile([BLOCK, NB_SHARD], dt.float32)
            nc.sync.dma_start(mag_d[:], mag_t[:])
            s_full = const_pool.tile([128, K_TILES * NB_SHARD], dt.float32)
            mag_3d = mag_d[:].rearrange("(kt r2) b -> kt r2 b", r2=2)
            for r2 in range(2):
                nc.sync.dma_start(
                    s_full[r2 * 64:(r2 + 1) * 64, :].rearrange(
                        "p (kt b) -> p kt b", b=NB_SHARD),
                    mag_3d[:, r2, :].unsqueeze(0).broadcast_to(
                        [64, K_TILES, NB_SHARD]),
                )

            # --- weights: stream sign, dequantize into resident fp32r tile
            w_all = w_pool.tile([128, K_TILES, N_SHARD], dt.float32r)
            for kt in range(K_TILES):
                st = sgn_pool.tile([128, N_SHARD], dt.float32)
                nc.sync.dma_start(st[:], sgn.ap()[kt * 128:(kt + 1) * 128, :])
                nc.vector.tensor_tensor(
                    out=w_all[:, kt, :].rearrange("p (b c) -> p b c", c=BLOCK),
                    in0=st[:].rearrange("p (b c) -> p b c", c=BLOCK),
                    in1=s_full[:, kt * NB_SHARD:(kt + 1) * NB_SHARD]
                        .unsqueeze(2).broadcast_to([128, NB_SHARD, BLOCK]),
                    op=mybir.AluOpType.mult,
                )

            # --- main loop over m-tiles
            xt_view = xt.ap().rearrange("(kt p) m -> p kt m", p=128)
            n_groups = K_TILES // XG
            for mt in range(M_TILES):
                groups = [
                    x_pool.tile([128, XG, 128], dt.float32r,
                                name=f"xg{g}", tag="xg")
                    for g in range(n_groups)
                ]
                for g in range(n_groups):
                    nc.sync.dma_start(
                        groups[g][:],
                        xt_view[:, g * XG:(g + 1) * XG,
                                mt * 128:(mt + 1) * 128])
                ps = ps_pool.tile([128, 2, 512], dt.float32)
                for kt in range(K_TILES):
                    for j in range(2):
                        nc.tensor.matmul(
                            ps[:, j, :],
                            groups[kt // XG][:, kt % XG, :],
                            w_all[:, kt, j * 512:(j + 1) * 512],
                            start=(kt == 0), stop=(kt == K_TILES - 1),
                        )
                ot = o_pool.tile([128, N_SHARD], dt.float32)
                nc.scalar.copy(
                    out=ot[:].rearrange("p (j n) -> p j n", j=2), in_=ps[:])
                nc.sync.dma_start(y.ap()[mt * 128:(mt + 1) * 128, :], ot[:])

    nc.compile()
    return nc


class _Runner:
    """Persistent compiled SPMD executable over the 8 axon cores."""

    def __init__(self):
        import jax
        from jax.sharding import Mesh, PartitionSpec
        from jax.experimental.shard_map import shard_map
        import concourse.mybir as mybir
        from concourse import bass2jax

        self.jax = jax
        nc = _build_module()
        self.nc = nc
        bass2jax.install_neuronx_cc_hook()

        partition_name = (nc.partition_id_tensor.name
                          if nc.partition_id_tensor else None)
        in_names = []
        out_names = []
        out_avals = []
        zero_outs = []
        for alloc in nc.m.functions[0].allocations:
            if not isinstance(alloc, mybir.MemoryLocationSet):
                continue
            name = alloc.memorylocations[0].name
            if alloc.kind == "ExternalInput":
                if name == partition_name:
                    continue
                in_names.append(name)
            elif alloc.kind == "ExternalOutput":
                out_names.append(name)
                shape = tuple(alloc.tensor_shape)
                dtype = mybir.dt.np(alloc.dtype)
                out_avals.append(jax.core.ShapedArray(shape, dtype))
                zero_outs.append(np.zeros(shape, dtype))
        self.in_names = list(in_names)
        self.out_names = out_names
        self.out_avals = out_avals
        n_params = len(in_names)
        all_names = in_names + out_names
        if partition_name is not None:
            all_names = all_names + [partition_name]

        def _body(*args):
            operands = list(args)
            if partition_name is not None:
                operands.append(bass2jax.partition_id_tensor())
            outs = bass2jax._bass_exec_p.bind(
                *operands,
                out_avals=tuple(out_avals),
                in_names=tuple(all_names),
                out_names=tuple(out_names),
                lowering_input_output_aliases=(),
                sim_require_finite=True,
                sim_require_nnan=True,
                nc=nc,
            )
            return tuple(outs)

        self._chain_body = _body
        devices = jax.devices()[:N_CORES]
        self.mesh = Mesh(np.asarray(devices), ("core",))
        n_outs = len(out_names)
        in_specs = (PartitionSpec("core"),) * (n_params + n_outs)
        out_specs = (PartitionSpec("core"),) * n_outs
        self._fn = jax.jit(
            shard_map(_body, mesh=self.mesh, in_specs=in_specs,
                      out_specs=out_specs, check_rep=False),
            keep_unused=True,
        )
        self.zero_outs = zero_outs
        self._zero_dev = None

    def put_inputs(self, in_maps):
        """Device-put concatenated per-core inputs; returns list of jax arrays."""
        from jax.sharding import NamedSharding, PartitionSpec
        sh = NamedSharding(self.mesh, PartitionSpec("core"))
        args = []
        for name in self.in_names:
            cat = np.concatenate([m[name] for m in in_maps], axis=0)
            args.append(self.jax.device_put(cat, sh))
        if self._zero_dev is None:
            self._zero_dev = [
                self.jax.device_put(
                    np.zeros((N_CORES * z.shape[0], *z.shape[1:]), z.dtype), sh)
                for z in self.zero_outs
            ]
        return args + self._zero_dev

    def run(self, args):
        outs = self._fn(*args)
        self.jax.block_until_ready(outs)
        return outs

    def split_outputs(self, outs):
        res = []
        for c in range(N_CORES):
            m = {}
            for i, name in enumerate(self.out_names):
                shape = self.out_avals[i].shape
                m[name] = np.asarray(outs[i]).reshape(N_CORES, *shape)[c]
            res.append(m)
        return res


def get_runner():
    global _RUNNER
    if _RUNNER is None:
        _RUNNER = _Runner()
    return _RUNNER


def make_in_maps(x, sign, block_scales):
    x = np.ascontiguousarray(x, dtype=np.float32)
    sign = np.ascontiguousarray(sign, dtype=np.float32)
    block_scales = np.ascontiguousarray(block_scales, dtype=np.float32)
    assert x.shape == (TOKENS, N_IN)
    assert sign.shape == (N_IN, N_OUT)
    assert block_scales.shape == (BLOCK, BLOCK)
    xt_halves = [
        np.ascontiguousarray(x[r * M_SHARD:(r + 1) * M_SHARD, :].T)
        for r in range(R_GROUPS)
    ]
    sgn_q = [
        np.ascontiguousarray(sign[:, q * N_SHARD:(q + 1) * N_SHARD])
        for q in range(Q_GROUPS)
    ]
    bs_q = [
        np.ascontiguousarray(block_scales[:, q * NB_SHARD:(q + 1) * NB_SHARD])
        for q in range(Q_GROUPS)
    ]
    in_maps = []
    for c in range(N_CORES):
        r, q = c // Q_GROUPS, c % Q_GROUPS
        in_maps.append({"xt": xt_halves[r], "sgn": sgn_q[q], "bs": bs_q[q]})
    return in_maps


def assemble(per_core_y):
    y = np.empty((TOKENS, N_OUT), dtype=np.float32)
    for c in range(N_CORES):
        r, q = c // Q_GROUPS, c % Q_GROUPS
        y[r * M_SHARD:(r + 1) * M_SHARD,
          q * N_SHARD:(q + 1) * N_SHARD] = per_core_y[c]
    return y


def kernel(x, sign, block_scales):
    runner = get_runner()
    in_maps = make_in_maps(x, sign, block_scales)
    args = runner.put_inputs(in_maps)
    outs = runner.run(args)
    per_core = runner.split_outputs(outs)
    return assemble([m["y"] for m in per_core])


if __name__ == "__main__":
    rng = np.random.default_rng(0)
    x = rng.standard_normal((TOKENS, N_IN), dtype=np.float32)
    sign = np.where(rng.standard_normal((N_IN, N_OUT)) >= 0, 1.0, -1.0).astype(np.float32)
    bs = rng.uniform(0.1, 1.0, (BLOCK, BLOCK)).astype(np.float32)
    t0 = time.perf_counter()
    out = kernel(x=x, sign=sign, block_scales=bs)
    print(f"kernel() wall: {time.perf_counter() - t0:.1f}s, out shape {out.shape}")
    mag = np.maximum(bs, 0) + 1e-6
    w = sign * np.repeat(np.repeat(mag, BLOCK, 0), BLOCK, 1)
    ref = x @ w
    l2 = np.linalg.norm(out - ref) / np.linalg.norm(ref)
    print(f"l2_rel vs fp32 numpy: {l2:.3e}")


# revision 49
# speedup vs baseline: 1.9718x; 1.0221x over previous
"""Trainium2 Bass kernel for nn_CTGCalibratedBinary.

Computes y = x @ (sign * expand64(relu(block_scales) + 1e-6)) for
x:(8192,4096), sign:(4096,4096), block_scales:(64,64), all fp32.

Sharding (8 cores): 2 token-groups x 4 out-col-groups.
  core c: r = c // 4 (token half), q = c % 4 (col quarter)
  per-core problem: y_c[4096, 1024] = x_r[4096, 4096] @ w[:, q*1024:(q+1)*1024]

Per-core kernel strategy (measured ~613 us/core steady-state on HW):
  - x is passed pre-transposed (xt = x_r.T, [K=4096, M=4096]) and declared
    float32r: the PE rounds fp32 -> 11-bit mantissa (RNE) on ingest
    (measured via identity-matmul probes), so matmuls run at full rate with
    ~1.4e-4 L2 error and no cast/rounding passes are needed anywhere.
  - sign shard is streamed once and dequantized on the vector engine into a
    fully SBUF-resident fp32r weight w_all[128, 32, 1024] (128 KiB/partition);
    the 64x64-block scales enter via a broadcast access pattern (stride-0
    inner dim), so dequant is a single tensor_tensor multiply per k-tile.
  - block scales: mag = relu(bs)+1e-6 fused in one tensor_scalar (max, add),
    then expanded to s_full[p, kt*16+b] = mag[2kt + p//64, b] with two
    broadcast DMAs through a DRAM scratch.
  - main loop over 32 m-tiles: x arrives as four [128, 8, 128] k-group tiles
    (finer DMA/dependency granularity), 64 matmuls accumulate K=4096 into
    2 PSUM banks (N=512 each, the fp32 moving-operand cap), ACT engine
    copies PSUM->SBUF, DMA out.

Perf notes from the measurement campaign (loop-in-NEFF differencing, since
axon wall-clock has ~30 ms transport noise): PE stream floor for the 2048
matmuls is ~530 us (259 ns/MM vs 213 ideal; the overhead is per-MM issue/
self-load cost, identical for bf16), x-streaming adds ~80 us (about half
pure HBM/port interference, half scheduling). Probed and rejected: psum-bank
alternation removal, stationary reuse ordering, --enable-ldw-opt=true,
transposed-output orientation (psum 8-bank cycling is worse), 1KB-line
slabs, DMA queue splitting, deeper prefetch. bf16 inputs are no faster on
this pattern and 16x less accurate; plain fp32 is 4x slower.
"""
import os
import sys
import time

for _p in ("/opt/trn_rl_repo",):
    if _p not in sys.path and os.path.isdir(_p):
        sys.path.insert(0, _p)

import numpy as np

TOKENS = 8192
N_IN = 4096
N_OUT = 4096
BLOCK = 64

N_CORES = 8
R_GROUPS = 2          # token groups
Q_GROUPS = 4          # out-col groups
M_SHARD = TOKENS // R_GROUPS      # 4096
N_SHARD = N_OUT // Q_GROUPS       # 1024
NB_SHARD = N_SHARD // BLOCK       # 16 col-blocks per core
K_TILES = N_IN // 128             # 32
M_TILES = M_SHARD // 128          # 32
XG = 8                            # k-tiles per x dependency-group tile
PS_BUFS = 4                       # psum tiles in flight (2 banks each)

_RUNNER = None


def _build_module(reps: int = 1):
    """Build the per-core Bass module. reps>1 wraps the body in a hardware
    For_i loop (identical iterations) -- used only for timing measurements."""
    import contextlib

    import concourse.mybir as mybir
    import concourse.tile as tile
    from concourse import bacc

    dt = mybir.dt
    nc = bacc.Bacc("TRN2", target_bir_lowering=False, debug=False,
                   num_devices=N_CORES)

    xt = nc.dram_tensor("xt", [N_IN, M_SHARD], dt.float32r, kind="ExternalInput")
    sgn = nc.dram_tensor("sgn", [N_IN, N_SHARD], dt.float32, kind="ExternalInput")
    bs = nc.dram_tensor("bs", [BLOCK, NB_SHARD], dt.float32, kind="ExternalInput")
    y = nc.dram_tensor("y", [M_SHARD, N_SHARD], dt.float32, kind="ExternalOutput")

    with tile.TileContext(nc) as tc:
        loop_ctx = (tc.For_i(0, reps, 1, hint_engines=(mybir.EngineType.PE,))
                    if reps > 1 else contextlib.nullcontext())
        with loop_ctx, \
             tc.tile_pool(name="const", bufs=1) as const_pool, \
             tc.tile_pool(name="dram", bufs=1, space="DRAM") as dram_pool, \
             tc.tile_pool(name="w", bufs=1) as w_pool, \
             tc.tile_pool(name="sgn", bufs=3) as sgn_pool, \
             tc.tile_pool(name="x", bufs=2 * (K_TILES // XG)) as x_pool, \
             tc.tile_pool(name="o", bufs=3) as o_pool, \
             tc.tile_pool(name="ps", bufs=PS_BUFS, space="PSUM") as ps_pool:

            # --- scales: mag = relu(bs) + 1e-6, expanded so that
            #     s_full[r2*64+p, kt*16+b] = mag[2*kt+r2, b]
            bs_t = const_pool.tile([BLOCK, NB_SHARD], dt.float32)
            nc.sync.dma_start(bs_t[:], bs.ap())
            mag_t = const_pool.tile([BLOCK, NB_SHARD], dt.float32)
            nc.vector.tensor_scalar(
                out=mag_t[:], in0=bs_t[:],
                scalar1=0.0, scalar2=1e-6,
                op0=mybir.AluOpType.max, op1=mybir.AluOpType.add,
            )
            mag_d = dram_pool.tile([BLOCK, NB_SHARD], dt.float32)
            nc.sync.dma_start(mag_d[:], mag_t[:])
            s_full = const_pool.tile([128, K_TILES * NB_SHARD], dt.float32)
            mag_3d = mag_d[:].rearrange("(kt r2) b -> kt r2 b", r2=2)
            for r2 in range(2):
                nc.sync.dma_start(
                    s_full[r2 * 64:(r2 + 1) * 64, :].rearrange(
                        "p (kt b) -> p kt b", b=NB_SHARD),
                    mag_3d[:, r2, :].unsqueeze(0).broadcast_to(
                        [64, K_TILES, NB_SHARD]),
                )

            # --- weights: stream sign, dequantize into resident fp32r tile
            w_all = w_pool.tile([128, K_TILES, N_SHARD], dt.float32r)
            for kt in range(K_TILES):
                st = sgn_pool.tile([128, N_SHARD], dt.float32)
                nc.sync.dma_start(st[:], sgn.ap()[kt * 128:(kt + 1) * 128, :])
                nc.vector.tensor_tensor(
                    out=w_all[:, kt, :].rearrange("p (b c) -> p b c", c=BLOCK),
                    in0=st[:].rearrange("p (b c) -> p b c", c=BLOCK),
                    in1=s_full[:, kt * NB_SHARD:(kt + 1) * NB_SHARD]
                        .unsqueeze(2).broadcast_to([128, NB_SHARD, BLOCK]),
                    op=mybir.AluOpType.mult,
                )

            # --- main loop over m-tiles
            xt_view = xt.ap().rearrange("(kt p) m -> p kt m", p=128)
            n_groups = K_TILES // XG
            for mt in range(M_TILES):
                groups = [
                    x_pool.tile([128, XG, 128], dt.float32r,
                                name=f"xg{g}", tag="xg")
                    for g in range(n_groups)
                ]
                for g in range(n_groups):
                    nc.sync.dma_start(
                        groups[g][:],
                        xt_view[:, g * XG:(g + 1) * XG,
                                mt * 128:(mt + 1) * 128])
                ps = ps_pool.tile([128, 2, 512], dt.float32)
                for kt in range(K_TILES):
                    for j in range(2):
                        nc.tensor.matmul(
                            ps[:, j, :],
                            groups[kt // XG][:, kt % XG, :],
                            w_all[:, kt, j * 512:(j + 1) * 512],
                            start=(kt == 0), stop=(kt == K_TILES - 1),
                        )
                ot = o_pool.tile([128, N_SHARD], dt.float32)
                nc.scalar.copy(
                    out=ot[:].rearrange("p (j n) -> p j n", j=2), in_=ps[:])
                nc.sync.dma_start(y.ap()[mt * 128:(mt + 1) * 128, :], ot[:])

    nc.compile()
    return nc


class _Runner:
    """Persistent compiled SPMD executable over the 8 axon cores."""

    def __init__(self):
        import jax
        from jax.sharding import Mesh, PartitionSpec
        from jax.experimental.shard_map import shard_map
        import concourse.mybir as mybir
        from concourse import bass2jax

        self.jax = jax
        nc = _build_module()
        self.nc = nc
        bass2jax.install_neuronx_cc_hook()

        partition_name = (nc.partition_id_tensor.name
                          if nc.partition_id_tensor else None)
        in_names = []
        out_names = []
        out_avals = []
        zero_outs = []
        for alloc in nc.m.functions[0].allocations:
            if not isinstance(alloc, mybir.MemoryLocationSet):
                continue
            name = alloc.memorylocations[0].name
            if alloc.kind == "ExternalInput":
                if name == partition_name:
                    continue
                in_names.append(name)
            elif alloc.kind == "ExternalOutput":
                out_names.append(name)
                shape = tuple(alloc.tensor_shape)
                dtype = mybir.dt.np(alloc.dtype)
                out_avals.append(jax.core.ShapedArray(shape, dtype))
                zero_outs.append(np.zeros(shape, dtype))
        self.in_names = list(in_names)
        self.out_names = out_names
        self.out_avals = out_avals
        n_params = len(in_names)
        all_names = in_names + out_names
        if partition_name is not None:
            all_names = all_names + [partition_name]

        def _body(*args):
            operands = list(args)
            if partition_name is not None:
                operands.append(bass2jax.partition_id_tensor())
            outs = bass2jax._bass_exec_p.bind(
                *operands,
                out_avals=tuple(out_avals),
                in_names=tuple(all_names),
                out_names=tuple(out_names),
                lowering_input_output_aliases=(),
                sim_require_finite=True,
                sim_require_nnan=True,
                nc=nc,
            )
            return tuple(outs)

        self._chain_body = _body
        devices = jax.devices()[:N_CORES]
        self.mesh = Mesh(np.asarray(devices), ("core",))
        n_outs = len(out_names)
        in_specs = (PartitionSpec("core"),) * (n_params + n_outs)
        out_specs = (PartitionSpec("core"),) * n_outs
        self._fn = jax.jit(
            shard_map(_body, mesh=self.mesh, in_specs=in_specs,
                      out_specs=out_specs, check_rep=False),
            keep_unused=True,
        )
        self.zero_outs = zero_outs
        self._zero_dev = None

    def put_inputs(self, in_maps):
        """Device-put concatenated per-core inputs; returns list of jax arrays."""
        from jax.sharding import NamedSharding, PartitionSpec
        sh = NamedSharding(self.mesh, PartitionSpec("core"))
        args = []
        for name in self.in_names:
            cat = np.concatenate([m[name] for m in in_maps], axis=0)
            args.append(self.jax.device_put(cat, sh))
        if self._zero_dev is None:
            self._zero_dev = [
                self.jax.device_put(
                    np.zeros((N_CORES * z.shape[0], *z.shape[1:]), z.dtype), sh)
                for z in self.zero_outs
            ]
        return args + self._zero_dev

    def run(self, args):
        outs = self._fn(*args)
        self.jax.block_until_ready(outs)
        return outs

    def split_outputs(self, outs):
        res = []
        for c in range(N_CORES):
            m = {}
            for i, name in enumerate(self.out_names):
                shape = self.out_avals[i].shape
                m[name] = np.asarray(outs[i]).reshape(N_CORES, *shape)[c]
            res.append(m)
        return res


def get_runner():
    global _RUNNER
    if _RUNNER is None:
        _RUNNER = _Runner()
    return _RUNNER


def make_in_maps(x, sign, block_scales):
    x = np.ascontiguousarray(x, dtype=np.float32)
    sign = np.ascontiguousarray(sign, dtype=np.float32)
    block_scales = np.ascontiguousarray(block_scales, dtype=np.float32)
    assert x.shape == (TOKENS, N_IN)
    assert sign.shape == (N_IN, N_OUT)
    assert block_scales.shape == (BLOCK, BLOCK)
    xt_halves = [
        np.ascontiguousarray(x[r * M_SHARD:(r + 1) * M_SHARD, :].T)
        for r in range(R_GROUPS)
    ]
    sgn_q = [
        np.ascontiguousarray(sign[:, q * N_SHARD:(q + 1) * N_SHARD])
        for q in range(Q_GROUPS)
    ]
    bs_q = [
        np.ascontiguousarray(block_scales[:, q * NB_SHARD:(q + 1) * NB_SHARD])
        for q in range(Q_GROUPS)
    ]
    in_maps = []
    for c in range(N_CORES):
        r, q = c // Q_GROUPS, c % Q_GROUPS
        in_maps.append({"xt": xt_halves[r], "sgn": sgn_q[q], "bs": bs_q[q]})
    return in_maps


def assemble(per_core_y):
    y = np.empty((TOKENS, N_OUT), dtype=np.float32)
    for c in range(N_CORES):
        r, q = c // Q_GROUPS, c % Q_GROUPS
        y[r * M_SHARD:(r + 1) * M_SHARD,
          q * N_SHARD:(q + 1) * N_SHARD] = per_core_y[c]
    return y


def kernel(x, sign, block_scales):
    runner = get_runner()
    in_maps = make_in_maps(x, sign, block_scales)
    args = runner.put_inputs(in_maps)
    outs = runner.run(args)
    per_core = runner.split_outputs(outs)
    return assemble([m["y"] for m in per_core])


if __name__ == "__main__":
    rng = np.random.default_rng(0)
    x = rng.standard_normal((TOKENS, N_IN), dtype=np.float32)
    sign = np.where(rng.standard_normal((N_IN, N_OUT)) >= 0, 1.0, -1.0).astype(np.float32)
    bs = rng.uniform(0.1, 1.0, (BLOCK, BLOCK)).astype(np.float32)
    t0 = time.perf_counter()
    out = kernel(x=x, sign=sign, block_scales=bs)
    print(f"kernel() wall: {time.perf_counter() - t0:.1f}s, out shape {out.shape}")
    mag = np.maximum(bs, 0) + 1e-6
    w = sign * np.repeat(np.repeat(mag, BLOCK, 0), BLOCK, 1)
    ref = x @ w
    l2 = np.linalg.norm(out - ref) / np.linalg.norm(ref)
    print(f"l2_rel vs fp32 numpy: {l2:.3e}")
